# revision 22
# baseline (speedup 1.0000x reference)
"""Tensor-parallel GQA attention layer for one TRN2 chip (8 NeuronCores).

Problem (hardcoded): x [1, 2048, 2048] f32, w_qkv [3072, 2048] f32,
w_o [2048, 2048] f32; NH=32 q heads, KVH=8 kv heads, D=64, causal, RoPE
(non-interleaved half rotation), GQA group = 4.

Sharding: core c owns kv head c and q heads [4c, 4c+4). Each core:
  1. qkv^T projection for its 384 w_qkv rows (x^T replicated, bf16)
  2. RoPE on q/k (transposed layout, cos/sin tables from host)
  3. causal flash attention for its 4 heads (scores^T orientation)
  4. AllToAll: head-sharded attn output -> seq-sharded
  5. o^T projection with full w_o for its 256 sequence rows
Host concatenates the 8 [256, 2048] f32 outputs.
"""

import math

import numpy as np
import ml_dtypes

import concourse.bass as bass
import concourse.bacc as bacc
import concourse.tile as tile
import concourse.mybir as mybir
from concourse import bass_utils

N_CORES = 8
S = 2048           # sequence length
H = 2048           # hidden size
NH, KVH, D = 32, 8, 64
G = NH // KVH      # q heads per kv head = 4
QH = NH // N_CORES # q heads per core = 4
F = QH * D + 2 * D # w_qkv rows per core = 384
SB = S // N_CORES  # seq rows per core = 256

BF16 = mybir.dt.bfloat16
F32 = mybir.dt.float32
NEG = -30000.0
SCALE = 1.0 / math.sqrt(D)

_NC_CACHE = {}


def _build_nc():
    nc = bacc.Bacc("TRN2", target_bir_lowering=False, debug=False,
                   num_devices=N_CORES)

    xt = nc.dram_tensor("xt", [H, S], BF16, kind="ExternalInput")
    wq = nc.dram_tensor("wq", [H, F], BF16, kind="ExternalInput")
    wo = nc.dram_tensor("wo", [H, H], BF16, kind="ExternalInput")
    cos4 = nc.dram_tensor("cos4", [128, S], F32, kind="ExternalInput")
    sin4 = nc.dram_tensor("sin4", [128, S], F32, kind="ExternalInput")
    tri = nc.dram_tensor("tri", [128, 128], BF16, kind="ExternalInput")
    ident = nc.dram_tensor("ident", [128, 128], BF16, kind="ExternalInput")
    out = nc.dram_tensor("out", [SB, H], F32, kind="ExternalOutput")

    KC = H // 128  # 16 contraction chunks

    with tile.TileContext(nc) as tc:
        with (
            tc.tile_pool(name="const", bufs=1) as const_pool,
            tc.tile_pool(name="persist", bufs=1) as persist,
            tc.tile_pool(name="dram", bufs=1, space="DRAM") as dram,
        ):
            # ---- constants / persistent tiles ----
            tri_sb = const_pool.tile([128, 128], BF16)
            nc.sync.dma_start(tri_sb[:], tri[:])
            id_sb = const_pool.tile([128, 128], BF16)
            nc.sync.dma_start(id_sb[:], ident[:])
            wq_sb = const_pool.tile([128, KC, F], BF16)
            nc.sync.dma_start(
                wq_sb[:], wq[:].rearrange("(kc p) f -> p kc f", p=128))

            # per-head Q^T, K^T (bf16, post-RoPE), V' = [V | ones]
            qt_sb = [persist.tile([D, S], BF16, name=f"qt{h}") for h in range(QH)]
            kt_sb = persist.tile([D, S], BF16)
            vp_sb = persist.tile([128, KC, D + 1], BF16)
            # normalized attention output (transposed), per head
            att_sb = [persist.tile([D, S], BF16, name=f"att{h}") for h in range(QH)]

            # ================= phase 1: qkv^T projection ================
            with (
                tc.tile_pool(name="qkv_sb", bufs=1) as qkv_pool,
                tc.tile_pool(name="cs_pool", bufs=1) as cs_pool,
                tc.tile_pool(name="tp_ps", bufs=2, space="PSUM") as tp_ps,
            ):
                cos_sb = cs_pool.tile([128, S], F32)
                nc.sync.dma_start(cos_sb[:], cos4[:])
                sin_sb = cs_pool.tile([128, S], F32)
                nc.sync.dma_start(sin_sb[:], sin4[:])
                # qkvT_sb[p, m, s]: m=0 q-first-halves, m=1 q-second-halves,
                # m=2 [k1(32) | k2(32) | v(64)]
                qkvT = qkv_pool.tile([128, 3, S], F32)

                with (
                    tc.tile_pool(name="xt_pool", bufs=1) as xt_pool,
                    tc.tile_pool(name="proj_ps", bufs=6, space="PSUM")
                    as proj_ps,
                ):
                    xt_sb = xt_pool.tile([128, KC, S], BF16)
                    for kc in range(KC):
                        nc.sync.dma_start(
                            xt_sb[:, kc, :], xt[kc * 128:(kc + 1) * 128, :])

                    for m in range(3):
                        ps = [proj_ps.tile([128, 512], F32, name=f"pp{m}_{nq}",
                                           tag="pp")
                              for nq in range(4)]
                        for kc in range(KC):
                            lhsT = wq_sb[:, kc, m * 128:(m + 1) * 128]
                            for nq in range(4):
                                nc.tensor.matmul(
                                    ps[nq][:], lhsT,
                                    xt_sb[:, kc, nq * 512:(nq + 1) * 512],
                                    start=(kc == 0), stop=(kc == KC - 1))
                        for nq in range(4):
                            nc.vector.tensor_copy(
                                qkvT[:, m, nq * 512:(nq + 1) * 512], ps[nq][:])

                # ============= phase 2: RoPE + V transpose =============
                rope_ctx = tc.tile_pool(name="rope_tmp", bufs=1)
                rope_tmp = rope_ctx.__enter__()
                # q rope over all 4 heads at once ([128, S] packed halves)
                q1 = qkvT[:, 0, :]
                q2 = qkvT[:, 1, :]
                t1 = rope_tmp.tile([128, S], F32, name="t1", tag="t", bufs=2)
                t2 = rope_tmp.tile([128, S], F32, name="t2", tag="t", bufs=2)
                q_onef = rope_tmp.tile([128, S], BF16, name="q_onef")
                q_twof = rope_tmp.tile([128, S], BF16, name="q_twof")
                nc.vector.tensor_mul(t1[:], q1, cos_sb[:])
                nc.vector.tensor_mul(t2[:], q2, sin_sb[:])
                nc.vector.tensor_sub(q_onef[:], t1[:], t2[:])
                t3 = rope_tmp.tile([128, S], F32, name="t3", tag="t", bufs=2)
                t4 = rope_tmp.tile([128, S], F32, name="t4", tag="t", bufs=2)
                nc.vector.tensor_mul(t3[:], q2, cos_sb[:])
                nc.vector.tensor_mul(t4[:], q1, sin_sb[:])
                nc.vector.tensor_add(q_twof[:], t3[:], t4[:])
                for h in range(QH):
                    sl = slice(h * 32, (h + 1) * 32)
                    nc.sync.dma_start(qt_sb[h][0:32, :], q_onef[sl, :])
                    nc.sync.dma_start(qt_sb[h][32:64, :], q_twof[sl, :])

                # k rope ([32, S] each half). Engine ops need matching start
                # partitions, so DMA k2 / v down to partition 0 first.
                k1 = qkvT[0:32, 2, :]
                k2_t = rope_tmp.tile([32, S], F32, name="k2_t")
                nc.sync.dma_start(k2_t[:], qkvT[32:64, 2, :])
                k2 = k2_t[:]
                u1 = rope_tmp.tile([32, S], F32, name="u1", tag="t", bufs=2,
                                   padded_shape=[128, S])
                u2 = rope_tmp.tile([32, S], F32, name="u2", tag="t", bufs=2,
                                   padded_shape=[128, S])
                k_onef = rope_tmp.tile([32, S], BF16, name="k_onef")
                k_twof = rope_tmp.tile([32, S], BF16, name="k_twof")
                nc.vector.tensor_mul(u1[:], k1, cos_sb[0:32, :])
                nc.vector.tensor_mul(u2[:], k2, sin_sb[0:32, :])
                nc.vector.tensor_sub(k_onef[:], u1[:], u2[:])
                nc.vector.tensor_mul(u1[:], k2, cos_sb[0:32, :])
                nc.vector.tensor_mul(u2[:], k1, sin_sb[0:32, :])
                nc.vector.tensor_add(k_twof[:], u1[:], u2[:])
                nc.sync.dma_start(kt_sb[0:32, :], k_onef[:])
                nc.sync.dma_start(kt_sb[32:64, :], k_twof[:])

                # V: cast + transpose into vp_sb[:, kb, 0:64]; ones col 64
                v_t = rope_tmp.tile([64, S], F32, name="v_t")
                nc.sync.dma_start(v_t[:], qkvT[64:128, 2, :])
                vt_bf = rope_tmp.tile([64, S], BF16, name="vt_bf")
                nc.vector.tensor_copy(vt_bf[:], v_t[:])
                nc.gpsimd.memset(vp_sb[:, :, D:D + 1], 1.0)
                for kb in range(KC):
                    tp = tp_ps.tile([128, D], BF16, name="tp")
                    nc.tensor.transpose(
                        tp[:], vt_bf[:, kb * 128:(kb + 1) * 128],
                        id_sb[0:64, 0:64])
                    nc.vector.tensor_copy(vp_sb[:, kb, 0:D], tp[:])
                rope_ctx.__exit__(None, None, None)

            # ================= phase 3: attention ================
            wo_pool_ctx = tc.tile_pool(name="wo_pool", bufs=1)
            wo_pool = wo_pool_ctx.__enter__()
            wo_sb = wo_pool.tile([128, KC, H], BF16)
            nc.sync.dma_start(
                wo_sb[:], wo[:].rearrange("(kc p) f -> p kc f", p=128))
            with (
                tc.tile_pool(name="sc_ps", bufs=2, space="PSUM") as sc_ps,
                tc.tile_pool(name="acc_ps", bufs=2, space="PSUM") as acc_ps,
                tc.tile_pool(name="pt_pool", bufs=3) as pt_pool,
                tc.tile_pool(name="div_pool", bufs=2) as div_pool,
            ):
                for h in range(QH):
                    for qh in range(2):
                        q_lo = qh * 1024
                        acc = acc_ps.tile([D + 1, 1024], F32, name="acc")
                        kb_max = 8 if qh == 0 else 16
                        for kb in range(kb_max):
                            k0 = kb * 128
                            s0 = max(q_lo, k0) - q_lo  # span start in half
                            scores = sc_ps.tile([128, 1024], F32, name="scores")
                            c = s0 // 512 * 512  # chunk-align
                            for c0 in range(c, 1024, 512):
                                nc.tensor.matmul(
                                    scores[:, c0:c0 + 512],
                                    kt_sb[:, k0:k0 + 128],
                                    qt_sb[h][:, q_lo + c0:q_lo + c0 + 512],
                                    start=True, stop=True)
                            pt = pt_pool.tile([128, 1024], BF16, name="pt")
                            nc.scalar.activation(
                                pt[:, s0:1024], scores[:, s0:1024],
                                mybir.ActivationFunctionType.Exp, scale=SCALE)
                            if k0 >= q_lo:  # diagonal: zero masked elements
                                nc.vector.tensor_mul(
                                    pt[:, s0:s0 + 128], pt[:, s0:s0 + 128],
                                    tri_sb[:])
                            for c0 in range(c, 1024, 512):
                                lo = max(c0, s0)
                                last_kb = (q_lo + c0) // 128 + 3
                                nc.tensor.matmul(
                                    acc[:, lo:c0 + 512],
                                    vp_sb[:, kb, :],
                                    pt[:, lo:c0 + 512],
                                    start=(kb == 0),
                                    stop=(kb == last_kb),
                                    skip_group_check=True)
                        # normalize: att = acc[0:64] * (1 / acc[64])
                        recip = div_pool.tile([D + 1, 1024], F32, name="recip")
                        nc.vector.reciprocal(recip[D:D + 1, :], acc[D:D + 1, :])
                        bcast = div_pool.tile([D, 1024], F32, name="bcast")
                        nc.sync.dma_start(
                            bcast[:],
                            recip[D:D + 1, :].unsqueeze(1)
                            .broadcast_to([1, D, 1024]))
                        nc.vector.tensor_mul(
                            att_sb[h][:, q_lo:q_lo + 1024], acc[0:D, :],
                            bcast[:])

            # ============ phase 4: AllToAll (head -> seq shard) ============
            a2a_in = dram.tile([N_CORES, QH * D, SB], BF16)
            a2a_out = dram.tile([N_CORES, QH * D, SB], BF16)
            for h in range(QH):
                nc.sync.dma_start(
                    a2a_in[:, h * D:(h + 1) * D, :].transpose([1, 0, 2]),
                    att_sb[h][:].rearrange("p (j q) -> p j q", j=N_CORES))
            nc.gpsimd.collective_compute(
                "AllToAll",
                mybir.AluOpType.bypass,
                replica_groups=[list(range(N_CORES))],
                ins=[a2a_in[:]],
                outs=[a2a_out[:]],
            )

            # ================= phase 5: o^T projection ================
            with (
                tc.tile_pool(name="attall", bufs=1) as attall_pool,
                tc.tile_pool(name="o_ps", bufs=8, space="PSUM") as o_ps,
                tc.tile_pool(name="o_sb", bufs=2) as o_sb_pool,
            ):
                attall = attall_pool.tile([128, KC, SB], BF16)
                for kc in range(KC):
                    nc.sync.dma_start(
                        attall[:, kc, :],
                        a2a_out[kc // 2, (kc % 2) * 128:(kc % 2) * 128 + 128, :])
                for sb in range(2):
                    po = [o_ps.tile([128, 512], F32, name=f"po{nf}", tag="po")
                          for nf in range(4)]
                    for kc in range(KC):
                        lhsT = attall[:, kc, sb * 128:(sb + 1) * 128]
                        for nf in range(4):
                            nc.tensor.matmul(
                                po[nf][:], lhsT,
                                wo_sb[:, kc, nf * 512:(nf + 1) * 512],
                                start=(kc == 0), stop=(kc == KC - 1))
                    o_out = o_sb_pool.tile([128, H], F32, name="o_out")
                    for nf in range(4):
                        nc.vector.tensor_copy(
                            o_out[:, nf * 512:(nf + 1) * 512], po[nf][:])
                    nc.sync.dma_start(out[sb * 128:(sb + 1) * 128, :], o_out[:])
            wo_pool_ctx.__exit__(None, None, None)

    nc.compile()
    return nc


def _host_inputs(x, w_qkv, w_o):
    """Build the 8 per-core input maps (host-side staging, bf16 weights)."""
    bf = ml_dtypes.bfloat16
    xt = np.ascontiguousarray(x.reshape(S, H).T).astype(bf)          # [H, S]
    wo_t = np.ascontiguousarray(w_o.T).astype(bf)                    # [H, H]

    # rope tables (match reference: inv_freq over even dims, outer with t)
    inv_freq = 1.0 / (10000.0 ** (np.arange(0, D, 2, dtype=np.float32) / D))
    t = np.arange(S, dtype=np.float32)
    freqs = np.outer(t, inv_freq)                                    # [S, 32]
    cos = np.cos(freqs).T.astype(np.float32)                         # [32, S]
    sin = np.sin(freqs).T.astype(np.float32)
    cos4 = np.ascontiguousarray(np.tile(cos, (4, 1)))                # [128, S]
    sin4 = np.ascontiguousarray(np.tile(sin, (4, 1)))

    # tri[k, q] = 1 if q >= k else 0 (valid part of diagonal 128-block)
    kk = np.arange(128)
    tri = (kk[None, :] >= kk[:, None]).astype(bf)                    # [128,128]
    ident = np.eye(128, dtype=bf)

    in_maps = []
    for c in range(N_CORES):
        cols = []
        # q first halves, q second halves (head-packed, 32 rows each)
        for half in range(2):
            for h in range(QH):
                g = (c * QH + h) * D + half * 32
                cols.append(w_qkv[g:g + 32, :])
        # k halves
        kbase = NH * D + c * D
        cols.append(w_qkv[kbase:kbase + 32, :])
        cols.append(w_qkv[kbase + 32:kbase + 64, :])
        # v
        vbase = NH * D + KVH * D + c * D
        cols.append(w_qkv[vbase:vbase + D, :])
        wq_c = np.concatenate(cols, axis=0)                          # [F, H]
        wq_ct = np.ascontiguousarray(wq_c.T).astype(bf)              # [H, F]
        in_maps.append({
            "xt": xt, "wq": wq_ct, "wo": wo_t,
            "cos4": cos4, "sin4": sin4, "tri": tri, "ident": ident,
        })
    return in_maps


def _run(x, w_qkv, w_o, trace=False):
    if "nc" not in _NC_CACHE:
        _NC_CACHE["nc"] = _build_nc()
    nc = _NC_CACHE["nc"]
    in_maps = _host_inputs(x, w_qkv, w_o)
    res = bass_utils.run_bass_kernel_spmd(
        nc, in_maps, core_ids=list(range(N_CORES)), trace=trace)
    out = np.concatenate(
        [res.results[c]["out"] for c in range(N_CORES)], axis=0)
    return out.reshape(1, S, H).astype(np.float32), res


def kernel(x, w_qkv, w_o):
    out, _ = _run(np.asarray(x), np.asarray(w_qkv), np.asarray(w_o))
    return out


# revision 23
# speedup vs baseline: 1.0868x; 1.0868x over previous
"""Tensor-parallel GQA attention layer for one TRN2 chip (8 NeuronCores).

Problem (hardcoded): x [1, 2048, 2048] f32, w_qkv [3072, 2048] f32,
w_o [2048, 2048] f32; NH=32 q heads, KVH=8 kv heads, D=64, causal, RoPE
(non-interleaved half rotation), GQA group = 4.

Sharding: core c owns kv head c and q heads [4c, 4c+4). Each core:
  1. qkv^T projection for its 384 w_qkv rows (x^T replicated, bf16)
  2. RoPE on q/k (transposed layout, cos/sin tables from host)
  3. causal flash attention for its 4 heads (scores^T orientation)
  4. AllToAll x2 (head-pair groups): head-sharded -> seq-sharded
  5. o^T projection with full w_o for its 256 sequence rows
Host concatenates the 8 [256, 2048] f32 outputs.
"""

import math

import numpy as np
import ml_dtypes

import concourse.bass as bass
import concourse.bacc as bacc
import concourse.tile as tile
import concourse.mybir as mybir
from concourse import bass_utils
from concourse.tile_legalize import tile_legalize as _tile_legalize_orig

N_CORES = 8
S = 2048           # sequence length
H = 2048           # hidden size
NH, KVH, D = 32, 8, 64
G = NH // KVH      # q heads per kv head = 4
QH = NH // N_CORES # q heads per core = 4
F = QH * D + 2 * D # w_qkv rows per core = 384
SB = S // N_CORES  # seq rows per core = 256

BF16 = mybir.dt.bfloat16
F32 = mybir.dt.float32
SCALE = 1.0 / math.sqrt(D)

_NC_CACHE = {}

# PE-stream instruction types that do not disturb loaded PE weights (they
# either run on other engines or are waits). Anything else resets the
# "weights currently loaded" tracking.
_LDW_SAFE = {
    "InstMatmult", "InstEventSemaphore", "InstTensorCopy", "InstTensorTensor",
    "InstActivation", "InstDMACopy", "InstReciprocal", "InstMemset",
    "InstCollectiveCompute", "InstTensorScalarPtr", "InstCustomDveAnt",
    "InstCopyPredicated", "InstTensorReduce", "InstIota",
}


def _tile_legalize_dedup_ldw(ordered_instructions_by_block, nc):
    """tile_legalize, then drop InstLdweights that reload the exact weights
    already resident in the PE array (consecutive duplicates with only
    non-PE-array-disturbing instructions in between). All matmul weights in
    this kernel are write-once tiles, so a duplicate load is always
    redundant."""
    out = _tile_legalize_orig(ordered_instructions_by_block, nc)
    for bb, insts in out.items():
        last_key = None
        kept = []
        for ins in insts:
            tn = type(ins).__name__
            if tn == "InstLdweights":
                key = repr(ins.ins[0])
                if key == last_key:
                    continue
                last_key = key
            elif tn in _LDW_SAFE:
                pass
            else:
                last_key = None
            kept.append(ins)
        out[bb] = kept
    return out


if getattr(tile.tile_legalize, "__name__", "") != "_tile_legalize_dedup_ldw":
    tile.tile_legalize = _tile_legalize_dedup_ldw


def _build_nc():
    nc = bacc.Bacc("TRN2", target_bir_lowering=False, debug=False,
                   num_devices=N_CORES)

    xt = nc.dram_tensor("xt", [H, S], BF16, kind="ExternalInput")
    wq = nc.dram_tensor("wq", [H, F], BF16, kind="ExternalInput")
    wo = nc.dram_tensor("wo", [H, H], BF16, kind="ExternalInput")
    cos4 = nc.dram_tensor("cos4", [128, S], F32, kind="ExternalInput")
    sin4 = nc.dram_tensor("sin4", [128, S], F32, kind="ExternalInput")
    tri = nc.dram_tensor("tri", [128, 128], BF16, kind="ExternalInput")
    ident = nc.dram_tensor("ident", [128, 128], BF16, kind="ExternalInput")
    out = nc.dram_tensor("out", [SB, H], F32, kind="ExternalOutput")

    KC = H // 128  # 16 contraction chunks

    with tile.TileContext(nc) as tc:
        with (
            tc.tile_pool(name="const", bufs=1) as const_pool,
            tc.tile_pool(name="persist", bufs=1) as persist,
            tc.tile_pool(name="dram", bufs=1, space="DRAM") as dram,
        ):
            # ---- constants / persistent tiles ----
            tri_sb = const_pool.tile([128, 128], BF16)
            nc.sync.dma_start(tri_sb[:], tri[:])
            id_sb = const_pool.tile([128, 128], BF16)
            nc.sync.dma_start(id_sb[:], ident[:])
            wq_sb = const_pool.tile([128, KC, F], BF16)
            nc.sync.dma_start(
                wq_sb[:], wq[:].rearrange("(kc p) f -> p kc f", p=128))

            # per-head Q^T, K^T (bf16, post-RoPE), V' = [V | ones]
            qt_sb = [persist.tile([D, S], BF16, name=f"qt{h}") for h in range(QH)]
            kt_sb = persist.tile([D, S], BF16)
            vp_sb = persist.tile([128, KC, D + 1], BF16)
            # normalized attention output (transposed), per head
            att_sb = [persist.tile([D, S], BF16, name=f"att{h}") for h in range(QH)]

            # ================= phase 1+2: qkv^T projection + RoPE ==========
            with (
                tc.tile_pool(name="qkv_sb", bufs=1) as qkv_pool,
                tc.tile_pool(name="cs_pool", bufs=1) as cs_pool,
                tc.tile_pool(name="tp_ps", bufs=2, space="PSUM") as tp_ps,
            ):
                cos_sb = cs_pool.tile([128, S], F32)
                nc.sync.dma_start(cos_sb[:], cos4[:])
                sin_sb = cs_pool.tile([128, S], F32)
                nc.sync.dma_start(sin_sb[:], sin4[:])
                # qkvT_sb[p, m, s]: m=0 q-first-halves, m=1 q-second-halves,
                # m=2 [k1(32) | k2(32) | v(64)]
                qkvT = qkv_pool.tile([128, 3, S], F32)

                rope_ctx = tc.tile_pool(name="rope_tmp", bufs=1)
                rope_tmp = rope_ctx.__enter__()

                with (
                    tc.tile_pool(name="xt_pool", bufs=1) as xt_pool,
                    tc.tile_pool(name="proj_ps", bufs=6, space="PSUM")
                    as proj_ps,
                ):
                    xt_tiles = []
                    for kc in range(KC):
                        t = xt_pool.tile([128, S], BF16, name=f"xtc{kc}",
                                         tag="xtc", bufs=KC)
                        nc.sync.dma_start(t[:], xt[kc * 128:(kc + 1) * 128, :])
                        xt_tiles.append(t)

                    # m=2 (k/v) first so k-rope + V transpose overlap the
                    # m=0/m=1 matmuls; q-rope then overlaps attention prep.
                    for m in (2, 0, 1):
                        ps = [proj_ps.tile([128, 512], F32, name=f"pp{m}_{nq}",
                                           tag="pp")
                              for nq in range(4)]
                        for kc in range(KC):
                            lhsT = wq_sb[:, kc, m * 128:(m + 1) * 128]
                            for nq in range(4):
                                nc.tensor.matmul(
                                    ps[nq][:], lhsT,
                                    xt_tiles[kc][:, nq * 512:(nq + 1) * 512],
                                    start=(kc == 0), stop=(kc == KC - 1))
                        for nq in range(4):
                            nc.vector.tensor_copy(
                                qkvT[:, m, nq * 512:(nq + 1) * 512], ps[nq][:])

                        if m != 2:
                            continue
                        # ---- k rope + V (depends only on m=2) ----
                        k1 = qkvT[0:32, 2, :]
                        k2_t = rope_tmp.tile([32, S], F32, name="k2_t")
                        nc.sync.dma_start(k2_t[:], qkvT[32:64, 2, :])
                        k2 = k2_t[:]
                        u1 = rope_tmp.tile([32, S], F32, name="u1", tag="t",
                                           bufs=2, padded_shape=[128, S])
                        u2 = rope_tmp.tile([32, S], F32, name="u2", tag="t",
                                           bufs=2, padded_shape=[128, S])
                        k_onef = rope_tmp.tile([32, S], BF16, name="k_onef")
                        k_twof = rope_tmp.tile([32, S], BF16, name="k_twof")
                        nc.vector.tensor_mul(u1[:], k1, cos_sb[0:32, :])
                        nc.vector.tensor_mul(u2[:], k2, sin_sb[0:32, :])
                        nc.vector.tensor_sub(k_onef[:], u1[:], u2[:])
                        nc.vector.tensor_mul(u1[:], k2, cos_sb[0:32, :])
                        nc.vector.tensor_mul(u2[:], k1, sin_sb[0:32, :])
                        nc.vector.tensor_add(k_twof[:], u1[:], u2[:])
                        nc.sync.dma_start(kt_sb[0:32, :], k_onef[:])
                        nc.sync.dma_start(kt_sb[32:64, :], k_twof[:])

                        v_t = rope_tmp.tile([64, S], F32, name="v_t")
                        nc.sync.dma_start(v_t[:], qkvT[64:128, 2, :])
                        vt_bf = rope_tmp.tile([64, S], BF16, name="vt_bf")
                        nc.vector.tensor_copy(vt_bf[:], v_t[:])
                        nc.gpsimd.memset(vp_sb[:, :, D:D + 1], 1.0)
                        for kb in range(KC):
                            tp = tp_ps.tile([128, D], BF16, name="tp")
                            nc.tensor.transpose(
                                tp[:], vt_bf[:, kb * 128:(kb + 1) * 128],
                                id_sb[0:64, 0:64])
                            nc.vector.tensor_copy(vp_sb[:, kb, 0:D], tp[:])

                # ---- q rope over all 4 heads at once ----
                q1 = qkvT[:, 0, :]
                q2 = qkvT[:, 1, :]
                t1 = rope_tmp.tile([128, S], F32, name="t1", tag="t", bufs=2)
                t2 = rope_tmp.tile([128, S], F32, name="t2", tag="t", bufs=2)
                q_onef = rope_tmp.tile([128, S], BF16, name="q_onef")
                q_twof = rope_tmp.tile([128, S], BF16, name="q_twof")
                nc.vector.tensor_mul(t1[:], q1, cos_sb[:])
                nc.vector.tensor_mul(t2[:], q2, sin_sb[:])
                nc.vector.tensor_sub(q_onef[:], t1[:], t2[:])
                t3 = rope_tmp.tile([128, S], F32, name="t3", tag="t", bufs=2)
                t4 = rope_tmp.tile([128, S], F32, name="t4", tag="t", bufs=2)
                nc.vector.tensor_mul(t3[:], q2, cos_sb[:])
                nc.vector.tensor_mul(t4[:], q1, sin_sb[:])
                nc.vector.tensor_add(q_twof[:], t3[:], t4[:])
                for h in range(QH):
                    sl = slice(h * 32, (h + 1) * 32)
                    nc.sync.dma_start(qt_sb[h][0:32, :], q_onef[sl, :])
                    nc.sync.dma_start(qt_sb[h][32:64, :], q_twof[sl, :])
                rope_ctx.__exit__(None, None, None)

            # ================= phase 3: attention ================
            wo_pool_ctx = tc.tile_pool(name="wo_pool", bufs=1)
            wo_pool = wo_pool_ctx.__enter__()
            wo_sb = wo_pool.tile([128, KC, H], BF16)
            nc.sync.dma_start(
                wo_sb[:], wo[:].rearrange("(kc p) f -> p kc f", p=128))

            # A2A ping-pong: group g carries heads {2g, 2g+1} (global feature
            # chunks kc = 2*core + g), fired as soon as its heads finish.
            a2a_in = [dram.tile([N_CORES, 2 * D, SB], BF16, name=f"a2ai{g}")
                      for g in range(2)]
            a2a_out = [dram.tile([N_CORES, 2 * D, SB], BF16, name=f"a2ao{g}")
                       for g in range(2)]

            def emit_a2a(g):
                for h in (2 * g, 2 * g + 1):
                    nc.sync.dma_start(
                        a2a_in[g][:, (h % 2) * D:(h % 2 + 1) * D, :]
                        .transpose([1, 0, 2]),
                        att_sb[h][:].rearrange("p (j q) -> p j q", j=N_CORES))
                nc.gpsimd.collective_compute(
                    "AllToAll",
                    mybir.AluOpType.bypass,
                    replica_groups=[list(range(N_CORES))],
                    ins=[a2a_in[g][:]],
                    outs=[a2a_out[g][:]],
                )

            with (
                tc.tile_pool(name="sc_ps", bufs=2, space="PSUM") as sc_ps,
                tc.tile_pool(name="acc_ps", bufs=2, space="PSUM") as acc_ps,
                tc.tile_pool(name="pt_pool", bufs=3) as pt_pool,
                tc.tile_pool(name="div_pool", bufs=2) as div_pool,
            ):
                for h in range(QH):
                    for qh in range(2):
                        q_lo = qh * 1024
                        acc = acc_ps.tile([D + 1, 1024], F32, name="acc")
                        kb_max = 8 if qh == 0 else 16
                        for kb in range(kb_max):
                            k0 = kb * 128
                            s0 = max(q_lo, k0) - q_lo  # span start in half
                            scores = sc_ps.tile([128, 1024], F32, name="scores")
                            c = s0 // 512 * 512  # chunk-align
                            for c0 in range(c, 1024, 512):
                                nc.tensor.matmul(
                                    scores[:, c0:c0 + 512],
                                    kt_sb[:, k0:k0 + 128],
                                    qt_sb[h][:, q_lo + c0:q_lo + c0 + 512],
                                    start=True, stop=True)
                            pt = pt_pool.tile([128, 1024], BF16, name="pt")
                            nc.scalar.activation(
                                pt[:, s0:1024], scores[:, s0:1024],
                                mybir.ActivationFunctionType.Exp, scale=SCALE)
                            if k0 >= q_lo:  # diagonal: zero masked elements
                                nc.vector.tensor_mul(
                                    pt[:, s0:s0 + 128], pt[:, s0:s0 + 128],
                                    tri_sb[:])
                            for c0 in range(c, 1024, 512):
                                lo = max(c0, s0)
                                last_kb = (q_lo + c0) // 128 + 3
                                nc.tensor.matmul(
                                    acc[:, lo:c0 + 512],
                                    vp_sb[:, kb, :],
                                    pt[:, lo:c0 + 512],
                                    start=(kb == 0),
                                    stop=(kb == last_kb),
                                    skip_group_check=True)
                        # normalize: att = acc[0:64] / acc[64].
                        # Reciprocal is free-dim-serial, so repack the 1x1024
                        # denominator row to [128, 8] first (DVE lanes).
                        den = div_pool.tile([D + 1, 1024], F32, name="den")
                        nc.vector.tensor_copy(den[D:D + 1, :], acc[D:D + 1, :])
                        dpk = div_pool.tile([128, 8], F32, name="dpk")
                        nc.sync.dma_start(
                            dpk[:],
                            den[D:D + 1, :].rearrange("a (p f) -> a p f",
                                                      p=128))
                        rpk = div_pool.tile([128, 8], F32, name="rpk")
                        nc.vector.reciprocal(rpk[:], dpk[:])
                        rrow = div_pool.tile([1, 1024], F32, name="rrow")
                        nc.sync.dma_start(
                            rrow[0:1, :].rearrange("a (p f) -> a p f", p=128),
                            rpk[:])
                        bcast = div_pool.tile([D, 1024], F32, name="bcast")
                        nc.sync.dma_start(
                            bcast[:],
                            rrow[0:1, :].unsqueeze(1)
                            .broadcast_to([1, D, 1024]))
                        nc.vector.tensor_mul(
                            att_sb[h][:, q_lo:q_lo + 1024], acc[0:D, :],
                            bcast[:])
                    if h == 1:
                        emit_a2a(0)
                emit_a2a(1)

            # ================= phase 5: o^T projection ================
            with (
                tc.tile_pool(name="attall", bufs=1) as attall_pool,
                tc.tile_pool(name="o_ps", bufs=8, space="PSUM") as o_ps,
                tc.tile_pool(name="o_sb", bufs=2) as o_sb_pool,
            ):
                attall = attall_pool.tile([128, KC, SB], BF16)
                for g in range(2):
                    for i in range(N_CORES):
                        nc.sync.dma_start(
                            attall[:, 2 * i + g, :], a2a_out[g][i, :, :])
                # g=0 chunks first: their matmuls only need the first A2A
                kc_order = [2 * i for i in range(N_CORES)] + \
                           [2 * i + 1 for i in range(N_CORES)]
                for sb in range(2):
                    po = [o_ps.tile([128, 512], F32, name=f"po{nf}", tag="po")
                          for nf in range(4)]
                    for idx, kc in enumerate(kc_order):
                        lhsT = attall[:, kc, sb * 128:(sb + 1) * 128]
                        for nf in range(4):
                            nc.tensor.matmul(
                                po[nf][:], lhsT,
                                wo_sb[:, kc, nf * 512:(nf + 1) * 512],
                                start=(idx == 0), stop=(idx == KC - 1))
                    o_out = o_sb_pool.tile([128, H], F32, name="o_out")
                    for nf in range(4):
                        nc.vector.tensor_copy(
                            o_out[:, nf * 512:(nf + 1) * 512], po[nf][:])
                    nc.sync.dma_start(out[sb * 128:(sb + 1) * 128, :], o_out[:])
            wo_pool_ctx.__exit__(None, None, None)

    nc.compile()
    return nc


def _host_inputs(x, w_qkv, w_o):
    """Build the 8 per-core input maps (host-side staging, bf16 weights)."""
    bf = ml_dtypes.bfloat16
    xt = np.ascontiguousarray(x.reshape(S, H).T).astype(bf)          # [H, S]
    wo_t = np.ascontiguousarray(w_o.T).astype(bf)                    # [H, H]

    # rope tables (match reference: inv_freq over even dims, outer with t)
    inv_freq = 1.0 / (10000.0 ** (np.arange(0, D, 2, dtype=np.float32) / D))
    t = np.arange(S, dtype=np.float32)
    freqs = np.outer(t, inv_freq)                                    # [S, 32]
    cos = np.cos(freqs).T.astype(np.float32)                         # [32, S]
    sin = np.sin(freqs).T.astype(np.float32)
    cos4 = np.ascontiguousarray(np.tile(cos, (4, 1)))                # [128, S]
    sin4 = np.ascontiguousarray(np.tile(sin, (4, 1)))

    # tri[k, q] = 1 if q >= k else 0 (valid part of diagonal 128-block)
    kk = np.arange(128)
    tri = (kk[None, :] >= kk[:, None]).astype(bf)                    # [128,128]
    ident = np.eye(128, dtype=bf)

    in_maps = []
    for c in range(N_CORES):
        cols = []
        # q first halves, q second halves (head-packed, 32 rows each)
        for half in range(2):
            for h in range(QH):
                g = (c * QH + h) * D + half * 32
                cols.append(w_qkv[g:g + 32, :])
        # k halves
        kbase = NH * D + c * D
        cols.append(w_qkv[kbase:kbase + 32, :])
        cols.append(w_qkv[kbase + 32:kbase + 64, :])
        # v
        vbase = NH * D + KVH * D + c * D
        cols.append(w_qkv[vbase:vbase + D, :])
        wq_c = np.concatenate(cols, axis=0)                          # [F, H]
        wq_ct = np.ascontiguousarray(wq_c.T).astype(bf)              # [H, F]
        in_maps.append({
            "xt": xt, "wq": wq_ct, "wo": wo_t,
            "cos4": cos4, "sin4": sin4, "tri": tri, "ident": ident,
        })
    return in_maps


def _run(x, w_qkv, w_o, trace=False):
    if "nc" not in _NC_CACHE:
        _NC_CACHE["nc"] = _build_nc()
    nc = _NC_CACHE["nc"]
    in_maps = _host_inputs(x, w_qkv, w_o)
    res = bass_utils.run_bass_kernel_spmd(
        nc, in_maps, core_ids=list(range(N_CORES)), trace=trace)
    out = np.concatenate(
        [res.results[c]["out"] for c in range(N_CORES)], axis=0)
    return out.reshape(1, S, H).astype(np.float32), res


def kernel(x, w_qkv, w_o):
    out, _ = _run(np.asarray(x), np.asarray(w_qkv), np.asarray(w_o))
    return out
